# revision 4
# baseline (speedup 1.0000x reference)
"""MoE (cosine-routed, top-k, 2-layer GELU FFN) on 8 Trainium2 NeuronCores.

Strategy (expert-parallel, per the sharding hint):
  - Host computes the (tiny) routing: cosine scores -> softmax -> top-k ->
    renormalized gate weights. ~34 MFLOP, negligible vs the 34 GFLOP FFN.
  - Tokens are dispatched by top-k expert id: core e receives the tokens
    routed to expert e (padded to capacity C), plus expert e's W1/b1/W2/b2.
  - Each core runs the 2-layer FFN in bf16 (fp32 PSUM accumulation) and
    scales each token's output by its gate weight on-device.
  - Host scatter-adds the (<= top_k) expert contributions per token.

Device layout per core (P = 128 partitions):
  GEMM1: hT[f, t] = sum_d W1[d, f] * xT[d, t]   (W1 tiles stationary)
         -> Gelu(. + b1) on ScalarE, cast to bf16
  GEMM2: yT[d, t] = sum_f W2[f, d] * hT[f, t]   (W2 tiles stationary)
         -> (. + b2) * gate on VectorE, fp32 out

All DRAM inputs are pre-arranged on the host into the exact SBUF layout
(partition-contiguous), so every DMA moves large contiguous per-partition
segments (>= 2KB bursts; the biggest transfers are single multi-MB DMAs).
A short run of dummy matmuls on a zeroed tile warms the PE HAM clock
(1.2 -> 2.4 GHz) while the first DMAs are in flight.
"""

import numpy as np
import ml_dtypes

P = 128
D_MODEL = 1024
D_FF = 2048
N_EXPERTS = 8
N_CORES = 8
N_WARMUP_MM = 34

_BF16 = ml_dtypes.bfloat16

_cache: dict = {}
last_results = None  # BassKernelResults of the most recent run (for profiling)


def _chunks(C):
    out = []
    c0 = 0
    while c0 < C:
        cw = min(512, C - c0)
        out.append((c0, cw))
        c0 += cw
    return out


def _build(C):
    """Build + compile the SPMD FFN kernel for capacity C (multiple of 32)."""
    import concourse.mybir as mybir
    from concourse import bacc
    from concourse.tile import TileContext

    D, F = D_MODEL, D_FF
    ND, NF = D // P, F // P

    nc = bacc.Bacc("TRN2", target_bir_lowering=False, debug=False,
                   enable_partition_id=False)

    # Host-pre-arranged layouts (see kernel() for the packing):
    #   xT:  [P, ND*C]    column d*C + t       = x[token t, d*P + part]
    #   w1:  [P, NF*ND*P] column f*ND*P + d*P + j = W1[d*P + part, f*P + j]
    #   w2:  [P, NF*D]    column f*D + j       = W2[f*P + part, j]
    xT_d = nc.dram_tensor("xT", [P, ND * C], mybir.dt.bfloat16, kind="ExternalInput")
    w1_d = nc.dram_tensor("w1", [P, NF * ND * P], mybir.dt.bfloat16,
                          kind="ExternalInput")
    w2_d = nc.dram_tensor("w2", [P, NF * D], mybir.dt.bfloat16, kind="ExternalInput")
    meta_d = nc.dram_tensor("meta", [P, NF + ND + C], mybir.dt.float32,
                            kind="ExternalInput")
    out_d = nc.dram_tensor("out", [D, C], mybir.dt.float32, kind="ExternalOutput")

    ck = _chunks(C)

    with TileContext(nc) as tc:
        with (
            tc.tile_pool(name="weights", bufs=1) as wp,
            tc.tile_pool(name="acts", bufs=1) as ap,
            tc.tile_pool(name="outs", bufs=4) as op,
            tc.tile_pool(name="psum", bufs=2, space="PSUM") as pp,
        ):
            # PSUM budget: 8 banks. Chunk 0 is double-buffered when the
            # bank budget allows (<= 2 chunks); extra chunks single-buffer.
            B0 = 2 if len(ck) <= 2 else 1
            psbufs = lambda ci: B0 if ci == 0 else 1

            # --- PE warm-up: dummy matmuls on a zeroed tile, no DMA deps.
            # The junk accumulator shares chunk 0's psum slot (used first).
            dummy = ap.tile([P, P], mybir.dt.bfloat16, tag="dummy")
            nc.vector.memset(dummy[:], 0.0)
            wps = pp.tile([P, P], mybir.dt.float32, tag="ps1_0", name="warm_ps",
                          bufs=B0)
            for _ in range(N_WARMUP_MM):
                nc.tensor.matmul(wps[:], dummy[:], dummy[:], start=True, stop=True)

            xt = ap.tile([P, ND * C], mybir.dt.bfloat16, tag="xt")
            w1t = wp.tile([P, NF * ND * P], mybir.dt.bfloat16, tag="w1")
            w2t = wp.tile([P, NF * D], mybir.dt.bfloat16, tag="w2")
            mt = wp.tile([P, NF + ND + C], mybir.dt.float32, tag="meta")
            b1t = mt[:, 0:NF]
            b2t = mt[:, NF : NF + ND]
            gt = mt[:, NF + ND : NF + ND + C]
            ht = ap.tile([P, NF * C], mybir.dt.bfloat16, tag="ht")

            # --- DMAs (all fully contiguous). Issue cost is ~0.65us per
            # dma_start on a HWDGE sequencer, and TRN2 has TWO of them
            # (SP=sync, Activation=scalar): split the issue load so the
            # critical-path inputs (x quarters, first W1 f-blocks) are in
            # flight as early as possible. Per-engine program order =
            # issue order. Scalar's queue opens late (ACT_TABLE_LOAD) and
            # must be free again by the first GELU, so it only gets a few.
            W1B = ND * P  # columns per W1 f-block
            XQ = (ND // 4) * C  # x quarter (2 d-tiles)
            nc.sync.dma_start(out=xt[:, :XQ], in_=xT_d[:, :XQ])
            nc.sync.dma_start(out=w1t[:, :W1B], in_=w1_d[:, :W1B])
            nc.sync.dma_start(out=w1t[:, W1B : 2 * W1B],
                              in_=w1_d[:, W1B : 2 * W1B])
            nc.sync.dma_start(out=xt[:, XQ : 2 * XQ], in_=xT_d[:, XQ : 2 * XQ])
            for f in range(2, NF):
                nc.sync.dma_start(out=w1t[:, f * W1B : (f + 1) * W1B],
                                  in_=w1_d[:, f * W1B : (f + 1) * W1B])
            nc.scalar.dma_start(out=mt[:], in_=meta_d[:])
            nc.scalar.dma_start(out=xt[:, 2 * XQ : 3 * XQ],
                                in_=xT_d[:, 2 * XQ : 3 * XQ])
            nc.scalar.dma_start(out=xt[:, 3 * XQ :], in_=xT_d[:, 3 * XQ :])
            NW2 = 4
            w2step = (NF // NW2) * D
            for i in range(NW2):
                eng = nc.scalar if i < 3 else nc.sync
                eng.dma_start(out=w2t[:, i * w2step : (i + 1) * w2step],
                              in_=w2_d[:, i * w2step : (i + 1) * w2step])

            # --- GEMM1 + GELU: hT[f*P:(f+1)*P, t].
            # Chunk loop inside the d-accumulation: each W1 stationary tile
            # serves len(ck) matmuls; LDWEIGHTS hides under the wide chunk.
            for f in range(NF):
                ps = [pp.tile([P, cw], mybir.dt.float32, tag=f"ps1_{ci}",
                              name=f"ps1_{f}_{ci}", bufs=psbufs(ci))
                      for ci, (c0, cw) in enumerate(ck)]
                for d in range(ND):
                    lhs = w1t[:, f * W1B + d * P : f * W1B + (d + 1) * P]
                    for ci, (c0, cw) in enumerate(ck):
                        nc.tensor.matmul(
                            ps[ci][:],
                            lhs,
                            xt[:, d * C + c0 : d * C + c0 + cw],
                            start=(d == 0),
                            stop=(d == ND - 1),
                        )
                for ci, (c0, cw) in enumerate(ck):
                    nc.scalar.activation(
                        ht[:, f * C + c0 : f * C + c0 + cw],
                        ps[ci][:],
                        mybir.ActivationFunctionType.Gelu,
                        bias=b1t[:, f : f + 1],
                    )

            # --- GEMM2 + bias + gate: yT[do*P:(do+1)*P, t].
            for do in range(ND):
                ps2 = [pp.tile([P, cw], mybir.dt.float32, tag=f"ps2_{ci}",
                               name=f"ps2_{do}_{ci}", bufs=psbufs(ci))
                       for ci, (c0, cw) in enumerate(ck)]
                for f in range(NF):
                    lhs = w2t[:, f * D + do * P : f * D + (do + 1) * P]
                    for ci, (c0, cw) in enumerate(ck):
                        nc.tensor.matmul(
                            ps2[ci][:],
                            lhs,
                            ht[:, f * C + c0 : f * C + c0 + cw],
                            start=(f == 0),
                            stop=(f == NF - 1),
                        )
                ot = op.tile([P, C], mybir.dt.float32, tag="ot",
                             name=f"ot_{do}")
                last = do == ND - 1
                for ci, (c0, cw) in enumerate(ck):
                    # Tail-latency trim: the final block's wide chunk is
                    # post-processed and DMA'd in halves on both HWDGE
                    # engines so the last transfer is small.
                    if last and cw > 256:
                        pieces = [(c0, cw // 2, nc.sync),
                                  (c0 + cw // 2, cw - cw // 2, nc.scalar)]
                    else:
                        pieces = [(c0, cw, nc.scalar if last else nc.sync)]
                    for p0, pw, eng in pieces:
                        nc.vector.scalar_tensor_tensor(
                            ot[:, p0 : p0 + pw],
                            ps2[ci][:, p0 - c0 : p0 - c0 + pw],
                            b2t[:, do : do + 1],
                            gt[:, p0 : p0 + pw],
                            op0=mybir.AluOpType.add,
                            op1=mybir.AluOpType.mult,
                        )
                        eng.dma_start(
                            out=out_d[do * P : (do + 1) * P, p0 : p0 + pw],
                            in_=ot[:, p0 : p0 + pw],
                        )

    nc.compile()
    return nc


def _get_kernel(C):
    if C not in _cache:
        _cache[C] = _build(C)
    return _cache[C]


def _run_spmd(nc, in_maps):
    """run_bass_kernel_spmd, robust to a BASS_TRACE env the image can't
    serve (missing antenv.axon_hooks / artifact upload): install a best-
    effort NTFF hook shim, and on a trace-path failure fall back to an
    untraced run."""
    import os
    from concourse.bass_utils import run_bass_kernel_spmd

    try:
        import antenv.axon_hooks  # noqa: F401
    except ImportError:
        import sys
        import types
        hook = None
        try:
            from trn_agent_boot.trn_boot import _ntff_profile_via_ctypes
            hook = _ntff_profile_via_ctypes("/opt/axon/libaxon_pjrt.so")
        except Exception:
            hook = None
        mod = types.ModuleType("antenv.axon_hooks")
        mod.get_axon_ntff_profile_hook = lambda: hook
        try:
            import antenv
            antenv.axon_hooks = mod
            sys.modules["antenv.axon_hooks"] = mod
        except ImportError:
            pass

    core_ids = list(range(N_CORES))
    try:
        return run_bass_kernel_spmd(nc, in_maps, core_ids)
    except Exception:
        if os.environ.get("BASS_NEVER_TRACE") == "1":
            raise
        os.environ["BASS_NEVER_TRACE"] = "1"
        try:
            return run_bass_kernel_spmd(nc, in_maps, core_ids)
        finally:
            del os.environ["BASS_NEVER_TRACE"]


def kernel(x, anchors, temperature, W1, b1, W2, b2, top_k):

    x = np.asarray(x)
    B, S, D = x.shape
    T = B * S
    E = np.asarray(anchors).shape[0]
    k = int(np.asarray(top_k))

    xf = np.ascontiguousarray(x.reshape(T, D), dtype=np.float32)

    # ---- routing on host (part of the dispatch decision) ----
    xn = xf / np.maximum(np.linalg.norm(xf, axis=-1, keepdims=True), 1e-8)
    an = np.asarray(anchors, dtype=np.float32)
    an = an / np.maximum(np.linalg.norm(an, axis=-1, keepdims=True), 1e-8)
    scores = (xn @ an.T) * abs(float(np.asarray(temperature)))
    scores -= scores.max(axis=-1, keepdims=True)
    probs = np.exp(scores)
    probs /= probs.sum(axis=-1, keepdims=True)
    topi = np.argsort(-probs, axis=-1, kind="stable")[:, :k]  # ties -> low idx
    topv = np.take_along_axis(probs, topi, axis=-1)
    gw = topv / (topv.sum(axis=-1, keepdims=True) + 1e-6)

    rows_per_e = []
    gates_per_e = []
    for e in range(E):
        mask = topi == e
        rows = np.nonzero(mask.any(axis=-1))[0]
        g = np.where(mask[rows], gw[rows], 0.0).sum(axis=-1).astype(np.float32)
        rows_per_e.append(rows)
        gates_per_e.append(g)

    max_count = max(len(r) for r in rows_per_e)
    C = max(64, -(-max_count // 32) * 32)
    nc = _get_kernel(C)

    # ---- per-core shards, pre-arranged into SBUF layouts ----
    x_bf = xf.astype(_BF16)
    ND, NF = D_MODEL // P, D_FF // P
    in_maps = []
    for e in range(N_CORES):
        rows = rows_per_e[e]
        n = len(rows)
        xT = np.zeros((P, ND * C), dtype=_BF16)
        # [P, ND, C] view: xT[p, d, t] = x[rows[t], d*P + p]
        xv = xT.reshape(P, ND, C)
        xv[:, :, :n] = x_bf[rows].reshape(n, ND, P).transpose(2, 1, 0)
        w1 = np.asarray(W1[e], dtype=np.float32).astype(_BF16)
        w1 = np.ascontiguousarray(
            w1.reshape(ND, P, NF, P).transpose(1, 2, 0, 3).reshape(P, NF * ND * P))
        w2 = np.asarray(W2[e], dtype=np.float32).astype(_BF16)
        w2 = np.ascontiguousarray(
            w2.reshape(NF, P, D_MODEL).transpose(1, 0, 2).reshape(P, NF * D_MODEL))
        meta = np.zeros((P, NF + ND + C), dtype=np.float32)
        meta[:, :NF] = np.asarray(b1[e], dtype=np.float32).reshape(NF, P).T
        meta[:, NF : NF + ND] = np.asarray(b2[e], dtype=np.float32).reshape(ND, P).T
        meta[:, NF + ND : NF + ND + n] = gates_per_e[e][None, :]
        in_maps.append({"xT": xT, "w1": w1, "w2": w2, "meta": meta})

    res = _run_spmd(nc, in_maps)
    global last_results
    last_results = res

    # ---- combine (scatter-add the gated expert outputs) ----
    out = np.zeros((T, D_MODEL), dtype=np.float32)
    for e in range(N_CORES):
        rows = rows_per_e[e]
        n = len(rows)
        if n:
            out[rows] += res.results[e]["out"][:, :n].T
    return out.reshape(B, S, D_MODEL)



# revision 5
# speedup vs baseline: 1.0962x; 1.0962x over previous
"""MoE (cosine-routed, top-k, 2-layer GELU FFN) on 8 Trainium2 NeuronCores.

Strategy (expert-parallel, per the sharding hint):
  - Host computes the (tiny) routing: cosine scores -> softmax -> top-k ->
    renormalized gate weights. ~34 MFLOP, negligible vs the 34 GFLOP FFN.
  - Tokens are dispatched by top-k expert id: core e receives the tokens
    routed to expert e (padded to capacity C), plus expert e's W1/b1/W2/b2.
  - Each core runs the 2-layer FFN in bf16 (fp32 PSUM accumulation) and
    scales each token's output by its gate weight on-device.
  - Host scatter-adds the (<= top_k) expert contributions per token.

Device layout per core (P = 128 partitions):
  GEMM1: hT[f, t] = sum_d W1[d, f] * xT[d, t]   (W1 tiles stationary)
         -> Gelu(. + b1) on ScalarE, cast to bf16
  GEMM2: yT[d, t] = sum_f W2[f, d] * hT[f, t]   (W2 tiles stationary)
         -> (. + b2) * gate on VectorE, fp32 out

All DRAM inputs are pre-arranged on the host into the exact SBUF layout
(partition-contiguous), so every DMA moves large contiguous per-partition
segments (>= 2KB bursts; the biggest transfers are single multi-MB DMAs).
A short run of dummy matmuls on a zeroed tile warms the PE HAM clock
(1.2 -> 2.4 GHz) while the first DMAs are in flight.
"""

import numpy as np
import ml_dtypes

P = 128
D_MODEL = 1024
D_FF = 2048
N_EXPERTS = 8
N_CORES = 8
N_WARMUP_MM = 34

_BF16 = ml_dtypes.bfloat16

_cache: dict = {}
last_results = None  # BassKernelResults of the most recent run (for profiling)


def _chunks(C):
    out = []
    c0 = 0
    while c0 < C:
        cw = min(512, C - c0)
        out.append((c0, cw))
        c0 += cw
    return out


def _build(C):
    """Build + compile the SPMD FFN kernel for capacity C (multiple of 32)."""
    import concourse.mybir as mybir
    from concourse import bacc
    from concourse.tile import TileContext

    D, F = D_MODEL, D_FF
    ND, NF = D // P, F // P

    nc = bacc.Bacc("TRN2", target_bir_lowering=False, debug=False,
                   enable_partition_id=False)

    # Host-pre-arranged layouts (see kernel() for the packing):
    #   xT:  [P, ND*C]    column d*C + t       = x[token t, d*P + part]
    #   w1:  [P, NF*ND*P] column f*ND*P + d*P + j = W1[d*P + part, f*P + j]
    #   w2:  [P, NF*D]    column f*D + j       = W2[f*P + part, j]
    xT_d = nc.dram_tensor("xT", [P, ND * C], mybir.dt.bfloat16, kind="ExternalInput")
    w1_d = nc.dram_tensor("w1", [P, NF * ND * P], mybir.dt.bfloat16,
                          kind="ExternalInput")
    w2_d = nc.dram_tensor("w2", [P, NF * D], mybir.dt.bfloat16, kind="ExternalInput")
    meta_d = nc.dram_tensor("meta", [P, NF + ND + C], mybir.dt.float32,
                            kind="ExternalInput")
    out_d = nc.dram_tensor("out", [D, C], mybir.dt.float32, kind="ExternalOutput")

    ck = _chunks(C)

    with TileContext(nc) as tc:
        with (
            tc.tile_pool(name="weights", bufs=1) as wp,
            tc.tile_pool(name="acts", bufs=1) as ap,
            tc.tile_pool(name="outs", bufs=4) as op,
            tc.tile_pool(name="psum", bufs=2, space="PSUM") as pp,
        ):
            # PSUM budget: 8 banks. Chunk 0 is double-buffered when the
            # bank budget allows (<= 2 chunks); extra chunks single-buffer.
            B0 = 2 if len(ck) <= 2 else 1
            psbufs = lambda ci: B0 if ci == 0 else 1

            # --- PE warm-up: dummy matmuls on a zeroed tile, no DMA deps.
            # The junk accumulator shares chunk 0's psum slot (used first).
            dummy = ap.tile([P, P], mybir.dt.bfloat16, tag="dummy")
            nc.vector.memset(dummy[:], 0.0)
            wps = pp.tile([P, P], mybir.dt.float32, tag="ps1_0", name="warm_ps",
                          bufs=B0)
            for _ in range(N_WARMUP_MM):
                nc.tensor.matmul(wps[:], dummy[:], dummy[:], start=True, stop=True)

            xt = ap.tile([P, ND * C], mybir.dt.bfloat16, tag="xt")
            w1t = wp.tile([P, NF * ND * P], mybir.dt.bfloat16, tag="w1")
            w2t = wp.tile([P, NF * D], mybir.dt.bfloat16, tag="w2")
            mt = wp.tile([P, NF + ND + C], mybir.dt.float32, tag="meta")
            b1t = mt[:, 0:NF]
            b2t = mt[:, NF : NF + ND]
            gt = mt[:, NF + ND : NF + ND + C]
            ht = ap.tile([P, NF * C], mybir.dt.bfloat16, tag="ht")

            # --- DMAs (all fully contiguous). Issue cost is ~0.65us per
            # dma_start on a HWDGE sequencer, and TRN2 has TWO of them
            # (SP=sync, Activation=scalar): split the issue load so the
            # critical-path inputs (x quarters, first W1 f-blocks) are in
            # flight as early as possible. Per-engine program order =
            # issue order. Scalar's queue opens late (ACT_TABLE_LOAD) and
            # must be free again by the first GELU, so it only gets a few.
            # Bandwidth priority is strict: x -> W1 blocks in f order -> W2.
            # Big low-priority transfers must NOT start early or they steal
            # HBM bandwidth from the critical path (measured +9us when W2
            # was issued early on scalar). Scalar only fronts the small
            # meta + two x quarters to cut the issue serialization.
            W1B = ND * P  # columns per W1 f-block
            XQ = (ND // 4) * C  # x quarter (2 d-tiles)
            nc.sync.dma_start(out=xt[:, :XQ], in_=xT_d[:, :XQ])
            nc.sync.dma_start(out=w1t[:, :W1B], in_=w1_d[:, :W1B])
            nc.sync.dma_start(out=xt[:, XQ : 2 * XQ], in_=xT_d[:, XQ : 2 * XQ])
            for f in range(1, NF):
                nc.sync.dma_start(out=w1t[:, f * W1B : (f + 1) * W1B],
                                  in_=w1_d[:, f * W1B : (f + 1) * W1B])
            nc.scalar.dma_start(out=mt[:], in_=meta_d[:])
            nc.scalar.dma_start(out=xt[:, 2 * XQ : 3 * XQ],
                                in_=xT_d[:, 2 * XQ : 3 * XQ])
            nc.scalar.dma_start(out=xt[:, 3 * XQ :], in_=xT_d[:, 3 * XQ :])
            NW2 = 4
            w2step = (NF // NW2) * D
            for i in range(NW2):
                nc.sync.dma_start(out=w2t[:, i * w2step : (i + 1) * w2step],
                                  in_=w2_d[:, i * w2step : (i + 1) * w2step])

            # --- GEMM1 + GELU: hT[f*P:(f+1)*P, t].
            # Chunk loop inside the d-accumulation: each W1 stationary tile
            # serves len(ck) matmuls; LDWEIGHTS hides under the wide chunk.
            for f in range(NF):
                ps = [pp.tile([P, cw], mybir.dt.float32, tag=f"ps1_{ci}",
                              name=f"ps1_{f}_{ci}", bufs=psbufs(ci))
                      for ci, (c0, cw) in enumerate(ck)]
                for d in range(ND):
                    lhs = w1t[:, f * W1B + d * P : f * W1B + (d + 1) * P]
                    for ci, (c0, cw) in enumerate(ck):
                        nc.tensor.matmul(
                            ps[ci][:],
                            lhs,
                            xt[:, d * C + c0 : d * C + c0 + cw],
                            start=(d == 0),
                            stop=(d == ND - 1),
                        )
                for ci, (c0, cw) in enumerate(ck):
                    nc.scalar.activation(
                        ht[:, f * C + c0 : f * C + c0 + cw],
                        ps[ci][:],
                        mybir.ActivationFunctionType.Gelu,
                        bias=b1t[:, f : f + 1],
                    )

            # --- GEMM2 + bias + gate: yT[do*P:(do+1)*P, t].
            for do in range(ND):
                ps2 = [pp.tile([P, cw], mybir.dt.float32, tag=f"ps2_{ci}",
                               name=f"ps2_{do}_{ci}", bufs=psbufs(ci))
                       for ci, (c0, cw) in enumerate(ck)]
                for f in range(NF):
                    lhs = w2t[:, f * D + do * P : f * D + (do + 1) * P]
                    for ci, (c0, cw) in enumerate(ck):
                        nc.tensor.matmul(
                            ps2[ci][:],
                            lhs,
                            ht[:, f * C + c0 : f * C + c0 + cw],
                            start=(f == 0),
                            stop=(f == NF - 1),
                        )
                ot = op.tile([P, C], mybir.dt.float32, tag="ot",
                             name=f"ot_{do}")
                last = do == ND - 1
                for ci, (c0, cw) in enumerate(ck):
                    # Tail-latency trim: the final block's wide chunk is
                    # post-processed and DMA'd in halves on both HWDGE
                    # engines so the last transfer is small.
                    if last and cw > 256:
                        pieces = [(c0, cw // 2, nc.sync),
                                  (c0 + cw // 2, cw - cw // 2, nc.scalar)]
                    else:
                        pieces = [(c0, cw, nc.scalar if last else nc.sync)]
                    for p0, pw, eng in pieces:
                        nc.vector.scalar_tensor_tensor(
                            ot[:, p0 : p0 + pw],
                            ps2[ci][:, p0 - c0 : p0 - c0 + pw],
                            b2t[:, do : do + 1],
                            gt[:, p0 : p0 + pw],
                            op0=mybir.AluOpType.add,
                            op1=mybir.AluOpType.mult,
                        )
                        eng.dma_start(
                            out=out_d[do * P : (do + 1) * P, p0 : p0 + pw],
                            in_=ot[:, p0 : p0 + pw],
                        )

    nc.compile()
    return nc


def _get_kernel(C):
    if C not in _cache:
        _cache[C] = _build(C)
    return _cache[C]


def _run_spmd(nc, in_maps):
    """run_bass_kernel_spmd, robust to a BASS_TRACE env the image can't
    serve (missing antenv.axon_hooks / artifact upload): install a best-
    effort NTFF hook shim, and on a trace-path failure fall back to an
    untraced run."""
    import os
    from concourse.bass_utils import run_bass_kernel_spmd

    try:
        import antenv.axon_hooks  # noqa: F401
    except ImportError:
        import sys
        import types
        hook = None
        try:
            from trn_agent_boot.trn_boot import _ntff_profile_via_ctypes
            hook = _ntff_profile_via_ctypes("/opt/axon/libaxon_pjrt.so")
        except Exception:
            hook = None
        mod = types.ModuleType("antenv.axon_hooks")
        mod.get_axon_ntff_profile_hook = lambda: hook
        try:
            import antenv
            antenv.axon_hooks = mod
            sys.modules["antenv.axon_hooks"] = mod
        except ImportError:
            pass

    core_ids = list(range(N_CORES))
    try:
        return run_bass_kernel_spmd(nc, in_maps, core_ids)
    except Exception:
        if os.environ.get("BASS_NEVER_TRACE") == "1":
            raise
        os.environ["BASS_NEVER_TRACE"] = "1"
        try:
            return run_bass_kernel_spmd(nc, in_maps, core_ids)
        finally:
            del os.environ["BASS_NEVER_TRACE"]


def kernel(x, anchors, temperature, W1, b1, W2, b2, top_k):

    x = np.asarray(x)
    B, S, D = x.shape
    T = B * S
    E = np.asarray(anchors).shape[0]
    k = int(np.asarray(top_k))

    xf = np.ascontiguousarray(x.reshape(T, D), dtype=np.float32)

    # ---- routing on host (part of the dispatch decision) ----
    xn = xf / np.maximum(np.linalg.norm(xf, axis=-1, keepdims=True), 1e-8)
    an = np.asarray(anchors, dtype=np.float32)
    an = an / np.maximum(np.linalg.norm(an, axis=-1, keepdims=True), 1e-8)
    scores = (xn @ an.T) * abs(float(np.asarray(temperature)))
    scores -= scores.max(axis=-1, keepdims=True)
    probs = np.exp(scores)
    probs /= probs.sum(axis=-1, keepdims=True)
    topi = np.argsort(-probs, axis=-1, kind="stable")[:, :k]  # ties -> low idx
    topv = np.take_along_axis(probs, topi, axis=-1)
    gw = topv / (topv.sum(axis=-1, keepdims=True) + 1e-6)

    rows_per_e = []
    gates_per_e = []
    for e in range(E):
        mask = topi == e
        rows = np.nonzero(mask.any(axis=-1))[0]
        g = np.where(mask[rows], gw[rows], 0.0).sum(axis=-1).astype(np.float32)
        rows_per_e.append(rows)
        gates_per_e.append(g)

    max_count = max(len(r) for r in rows_per_e)
    C = max(64, -(-max_count // 32) * 32)
    nc = _get_kernel(C)

    # ---- per-core shards, pre-arranged into SBUF layouts ----
    x_bf = xf.astype(_BF16)
    ND, NF = D_MODEL // P, D_FF // P
    in_maps = []
    for e in range(N_CORES):
        rows = rows_per_e[e]
        n = len(rows)
        xT = np.zeros((P, ND * C), dtype=_BF16)
        # [P, ND, C] view: xT[p, d, t] = x[rows[t], d*P + p]
        xv = xT.reshape(P, ND, C)
        xv[:, :, :n] = x_bf[rows].reshape(n, ND, P).transpose(2, 1, 0)
        w1 = np.asarray(W1[e], dtype=np.float32).astype(_BF16)
        w1 = np.ascontiguousarray(
            w1.reshape(ND, P, NF, P).transpose(1, 2, 0, 3).reshape(P, NF * ND * P))
        w2 = np.asarray(W2[e], dtype=np.float32).astype(_BF16)
        w2 = np.ascontiguousarray(
            w2.reshape(NF, P, D_MODEL).transpose(1, 0, 2).reshape(P, NF * D_MODEL))
        meta = np.zeros((P, NF + ND + C), dtype=np.float32)
        meta[:, :NF] = np.asarray(b1[e], dtype=np.float32).reshape(NF, P).T
        meta[:, NF : NF + ND] = np.asarray(b2[e], dtype=np.float32).reshape(ND, P).T
        meta[:, NF + ND : NF + ND + n] = gates_per_e[e][None, :]
        in_maps.append({"xT": xT, "w1": w1, "w2": w2, "meta": meta})

    res = _run_spmd(nc, in_maps)
    global last_results
    last_results = res

    # ---- combine (scatter-add the gated expert outputs) ----
    out = np.zeros((T, D_MODEL), dtype=np.float32)
    for e in range(N_CORES):
        rows = rows_per_e[e]
        n = len(rows)
        if n:
            out[rows] += res.results[e]["out"][:, :n].T
    return out.reshape(B, S, D_MODEL)



# revision 8
# speedup vs baseline: 1.1374x; 1.0376x over previous
"""MoE (cosine-routed, top-k, 2-layer GELU FFN) on 8 Trainium2 NeuronCores.

Strategy (expert-parallel, per the sharding hint):
  - Host computes the (tiny) routing: cosine scores -> softmax -> top-k ->
    renormalized gate weights. ~34 MFLOP, negligible vs the 34 GFLOP FFN.
  - Tokens are dispatched by top-k expert id: core e receives the tokens
    routed to expert e (padded to capacity C), plus expert e's W1/b1/W2/b2.
  - Each core runs the 2-layer FFN in bf16 (fp32 PSUM accumulation) and
    scales each token's output by its gate weight on-device.
  - Host scatter-adds the (<= top_k) expert contributions per token.

Device layout per core (P = 128 partitions):
  GEMM1: hT[f, t] = sum_d W1[d, f] * xT[d, t]   (W1 tiles stationary)
         -> Gelu(. + b1) on ScalarE, cast to bf16
  GEMM2: yT[d, t] = sum_f W2[f, d] * hT[f, t]   (W2 tiles stationary)
         -> (. + b2) * gate on VectorE, fp32 out

All DRAM inputs are pre-arranged on the host into the exact SBUF layout
(partition-contiguous), so every DMA moves large contiguous per-partition
segments (>= 2KB bursts; the biggest transfers are single multi-MB DMAs).
A short run of dummy matmuls on a zeroed tile warms the PE HAM clock
(1.2 -> 2.4 GHz) while the first DMAs are in flight.
"""

import numpy as np
import ml_dtypes

P = 128
D_MODEL = 1024
D_FF = 2048
N_EXPERTS = 8
N_CORES = 8
N_WARMUP_MM = 52

_BF16 = ml_dtypes.bfloat16

_cache: dict = {}
last_results = None  # BassKernelResults of the most recent run (for profiling)


def _chunks(C):
    out = []
    c0 = 0
    while c0 < C:
        cw = min(512, C - c0)
        out.append((c0, cw))
        c0 += cw
    return out


def _build(C):
    """Build + compile the SPMD FFN kernel for capacity C (multiple of 32)."""
    import concourse.mybir as mybir
    from concourse import bacc
    from concourse.tile import TileContext

    D, F = D_MODEL, D_FF
    ND, NF = D // P, F // P

    nc = bacc.Bacc("TRN2", target_bir_lowering=False, debug=False,
                   enable_partition_id=False)

    # Host-pre-arranged layouts (see kernel() for the packing):
    #   xT:  [P, ND*C]    column d*C + t       = x[token t, d*P + part]
    #   w1:  [P, NF*ND*P] column f*ND*P + d*P + j = W1[d*P + part, f*P + j]
    #   w2:  [P, NF*D]    column f*D + j       = W2[f*P + part, j]
    xT_d = nc.dram_tensor("xT", [P, ND * C], mybir.dt.bfloat16, kind="ExternalInput")
    w1_d = nc.dram_tensor("w1", [P, NF * ND * P], mybir.dt.bfloat16,
                          kind="ExternalInput")
    w2_d = nc.dram_tensor("w2", [P, NF * D], mybir.dt.bfloat16, kind="ExternalInput")
    meta_d = nc.dram_tensor("meta", [P, NF + ND + C], mybir.dt.float32,
                            kind="ExternalInput")
    out_d = nc.dram_tensor("out", [D, C], mybir.dt.float32, kind="ExternalOutput")

    ck = _chunks(C)

    with TileContext(nc) as tc:
        with (
            tc.tile_pool(name="weights", bufs=1) as wp,
            tc.tile_pool(name="acts", bufs=1) as ap,
            tc.tile_pool(name="outs", bufs=4) as op,
            tc.tile_pool(name="psum", bufs=2, space="PSUM") as pp,
        ):
            # PSUM budget: 8 banks. Chunk 0 is double-buffered when the
            # bank budget allows (<= 2 chunks); extra chunks single-buffer.
            B0 = 2 if len(ck) <= 2 else 1
            psbufs = lambda ci: B0 if ci == 0 else 1

            # --- PE warm-up: dummy matmuls on a zeroed tile, no DMA deps.
            # The junk accumulator shares chunk 0's psum slot (used first).
            dummy = ap.tile([P, P], mybir.dt.bfloat16, tag="dummy")
            nc.vector.memset(dummy[:], 0.0)
            wps = pp.tile([P, P], mybir.dt.float32, tag="ps1_0", name="warm_ps",
                          bufs=B0)
            for _ in range(N_WARMUP_MM):
                nc.tensor.matmul(wps[:], dummy[:], dummy[:], start=True, stop=True)

            xt = ap.tile([P, ND * C], mybir.dt.bfloat16, tag="xt")
            w1t = wp.tile([P, NF * ND * P], mybir.dt.bfloat16, tag="w1")
            w2t = wp.tile([P, NF * D], mybir.dt.bfloat16, tag="w2")
            mt = wp.tile([P, NF + ND + C], mybir.dt.float32, tag="meta")
            b1t = mt[:, 0:NF]
            b2t = mt[:, NF : NF + ND]
            gt = mt[:, NF + ND : NF + ND + C]
            ht = ap.tile([P, NF * C], mybir.dt.bfloat16, tag="ht")

            # --- DMAs (all fully contiguous). Issue cost is ~0.65us per
            # dma_start on a HWDGE sequencer, and TRN2 has TWO of them
            # (SP=sync, Activation=scalar): split the issue load so the
            # critical-path inputs (x quarters, first W1 f-blocks) are in
            # flight as early as possible. Per-engine program order =
            # issue order. Scalar's queue opens late (ACT_TABLE_LOAD) and
            # must be free again by the first GELU, so it only gets a few.
            # Bandwidth priority is strict: x -> W1 blocks in f order -> W2.
            # The 16 DMA engines round-robin across live transfers with no
            # priority, so big low-priority transfers must NOT be issued
            # early or they steal HBM bandwidth from the critical path
            # (measured +9us when W2 was fronted on the scalar queue).
            # Scalar gets only the tiny meta tensor; everything else keeps
            # baseline's proven FIFO order on sync.
            W1B = ND * P  # columns per W1 f-block
            XH = (ND // 2) * C
            nc.sync.dma_start(out=xt[:, :XH], in_=xT_d[:, :XH])
            nc.sync.dma_start(out=w1t[:, :W1B], in_=w1_d[:, :W1B])
            nc.sync.dma_start(out=xt[:, XH:], in_=xT_d[:, XH:])
            for f in range(1, NF):
                nc.sync.dma_start(out=w1t[:, f * W1B : (f + 1) * W1B],
                                  in_=w1_d[:, f * W1B : (f + 1) * W1B])
            nc.scalar.dma_start(out=mt[:], in_=meta_d[:])
            NW2 = 4
            w2step = (NF // NW2) * D
            for i in range(NW2):
                nc.sync.dma_start(out=w2t[:, i * w2step : (i + 1) * w2step],
                                  in_=w2_d[:, i * w2step : (i + 1) * w2step])

            # --- GEMM1 + GELU: hT[f*P:(f+1)*P, t].
            # Chunk loop inside the d-accumulation: each W1 stationary tile
            # serves len(ck) matmuls; LDWEIGHTS hides under the wide chunk.
            for f in range(NF):
                ps = [pp.tile([P, cw], mybir.dt.float32, tag=f"ps1_{ci}",
                              name=f"ps1_{f}_{ci}", bufs=psbufs(ci))
                      for ci, (c0, cw) in enumerate(ck)]
                for d in range(ND):
                    lhs = w1t[:, f * W1B + d * P : f * W1B + (d + 1) * P]
                    for ci, (c0, cw) in enumerate(ck):
                        nc.tensor.matmul(
                            ps[ci][:],
                            lhs,
                            xt[:, d * C + c0 : d * C + c0 + cw],
                            start=(d == 0),
                            stop=(d == ND - 1),
                        )
                for ci, (c0, cw) in enumerate(ck):
                    nc.scalar.activation(
                        ht[:, f * C + c0 : f * C + c0 + cw],
                        ps[ci][:],
                        mybir.ActivationFunctionType.Gelu,
                        bias=b1t[:, f : f + 1],
                    )

            # --- GEMM2 + bias + gate: yT[do*P:(do+1)*P, t].
            for do in range(ND):
                ps2 = [pp.tile([P, cw], mybir.dt.float32, tag=f"ps2_{ci}",
                               name=f"ps2_{do}_{ci}", bufs=psbufs(ci))
                       for ci, (c0, cw) in enumerate(ck)]
                for f in range(NF):
                    lhs = w2t[:, f * D + do * P : f * D + (do + 1) * P]
                    for ci, (c0, cw) in enumerate(ck):
                        nc.tensor.matmul(
                            ps2[ci][:],
                            lhs,
                            ht[:, f * C + c0 : f * C + c0 + cw],
                            start=(f == 0),
                            stop=(f == NF - 1),
                        )
                ot = op.tile([P, C], mybir.dt.float32, tag="ot",
                             name=f"ot_{do}")
                last = do == ND - 1
                # Tail-latency trim: for the final block, drain the narrow
                # chunk first and split the wide chunk across both HWDGE
                # engines so the last exposed transfer is small.
                order = list(enumerate(ck))
                if last:
                    order = order[::-1]
                for ci, (c0, cw) in order:
                    if last and cw > 256:
                        pieces = [(c0, cw // 2, nc.sync),
                                  (c0 + cw // 2, cw - cw // 2, nc.scalar)]
                    else:
                        pieces = [(c0, cw, nc.scalar if last else nc.sync)]
                    for p0, pw, eng in pieces:
                        nc.vector.scalar_tensor_tensor(
                            ot[:, p0 : p0 + pw],
                            ps2[ci][:, p0 - c0 : p0 - c0 + pw],
                            b2t[:, do : do + 1],
                            gt[:, p0 : p0 + pw],
                            op0=mybir.AluOpType.add,
                            op1=mybir.AluOpType.mult,
                        )
                        eng.dma_start(
                            out=out_d[do * P : (do + 1) * P, p0 : p0 + pw],
                            in_=ot[:, p0 : p0 + pw],
                        )

    nc.compile()
    return nc


def _get_kernel(C):
    if C not in _cache:
        _cache[C] = _build(C)
    return _cache[C]


def _run_spmd(nc, in_maps):
    """run_bass_kernel_spmd, robust to a BASS_TRACE env the image can't
    serve (missing antenv.axon_hooks / artifact upload): install a best-
    effort NTFF hook shim, and on a trace-path failure fall back to an
    untraced run."""
    import os
    from concourse.bass_utils import run_bass_kernel_spmd

    try:
        import antenv.axon_hooks  # noqa: F401
    except ImportError:
        import sys
        import types
        hook = None
        try:
            from trn_agent_boot.trn_boot import _ntff_profile_via_ctypes
            hook = _ntff_profile_via_ctypes("/opt/axon/libaxon_pjrt.so")
        except Exception:
            hook = None
        mod = types.ModuleType("antenv.axon_hooks")
        mod.get_axon_ntff_profile_hook = lambda: hook
        try:
            import antenv
            antenv.axon_hooks = mod
            sys.modules["antenv.axon_hooks"] = mod
        except ImportError:
            pass

    core_ids = list(range(N_CORES))
    try:
        return run_bass_kernel_spmd(nc, in_maps, core_ids)
    except Exception:
        if os.environ.get("BASS_NEVER_TRACE") == "1":
            raise
        os.environ["BASS_NEVER_TRACE"] = "1"
        try:
            return run_bass_kernel_spmd(nc, in_maps, core_ids)
        finally:
            del os.environ["BASS_NEVER_TRACE"]


def kernel(x, anchors, temperature, W1, b1, W2, b2, top_k):

    x = np.asarray(x)
    B, S, D = x.shape
    T = B * S
    E = np.asarray(anchors).shape[0]
    k = int(np.asarray(top_k))

    xf = np.ascontiguousarray(x.reshape(T, D), dtype=np.float32)

    # ---- routing on host (part of the dispatch decision) ----
    xn = xf / np.maximum(np.linalg.norm(xf, axis=-1, keepdims=True), 1e-8)
    an = np.asarray(anchors, dtype=np.float32)
    an = an / np.maximum(np.linalg.norm(an, axis=-1, keepdims=True), 1e-8)
    scores = (xn @ an.T) * abs(float(np.asarray(temperature)))
    scores -= scores.max(axis=-1, keepdims=True)
    probs = np.exp(scores)
    probs /= probs.sum(axis=-1, keepdims=True)
    topi = np.argsort(-probs, axis=-1, kind="stable")[:, :k]  # ties -> low idx
    topv = np.take_along_axis(probs, topi, axis=-1)
    gw = topv / (topv.sum(axis=-1, keepdims=True) + 1e-6)

    rows_per_e = []
    gates_per_e = []
    for e in range(E):
        mask = topi == e
        rows = np.nonzero(mask.any(axis=-1))[0]
        g = np.where(mask[rows], gw[rows], 0.0).sum(axis=-1).astype(np.float32)
        rows_per_e.append(rows)
        gates_per_e.append(g)

    max_count = max(len(r) for r in rows_per_e)
    C = max(64, -(-max_count // 32) * 32)
    nc = _get_kernel(C)

    # ---- per-core shards, pre-arranged into SBUF layouts ----
    x_bf = xf.astype(_BF16)
    ND, NF = D_MODEL // P, D_FF // P
    in_maps = []
    for e in range(N_CORES):
        rows = rows_per_e[e]
        n = len(rows)
        xT = np.zeros((P, ND * C), dtype=_BF16)
        # [P, ND, C] view: xT[p, d, t] = x[rows[t], d*P + p]
        xv = xT.reshape(P, ND, C)
        xv[:, :, :n] = x_bf[rows].reshape(n, ND, P).transpose(2, 1, 0)
        w1 = np.asarray(W1[e], dtype=np.float32).astype(_BF16)
        w1 = np.ascontiguousarray(
            w1.reshape(ND, P, NF, P).transpose(1, 2, 0, 3).reshape(P, NF * ND * P))
        w2 = np.asarray(W2[e], dtype=np.float32).astype(_BF16)
        w2 = np.ascontiguousarray(
            w2.reshape(NF, P, D_MODEL).transpose(1, 0, 2).reshape(P, NF * D_MODEL))
        meta = np.zeros((P, NF + ND + C), dtype=np.float32)
        meta[:, :NF] = np.asarray(b1[e], dtype=np.float32).reshape(NF, P).T
        meta[:, NF : NF + ND] = np.asarray(b2[e], dtype=np.float32).reshape(ND, P).T
        meta[:, NF + ND : NF + ND + n] = gates_per_e[e][None, :]
        in_maps.append({"xT": xT, "w1": w1, "w2": w2, "meta": meta})

    res = _run_spmd(nc, in_maps)
    global last_results
    last_results = res

    # ---- combine (scatter-add the gated expert outputs) ----
    out = np.zeros((T, D_MODEL), dtype=np.float32)
    for e in range(N_CORES):
        rows = rows_per_e[e]
        n = len(rows)
        if n:
            out[rows] += res.results[e]["out"][:, :n].T
    return out.reshape(B, S, D_MODEL)



# revision 9
# speedup vs baseline: 1.1563x; 1.0167x over previous
"""MoE (cosine-routed, top-k, 2-layer GELU FFN) on 8 Trainium2 NeuronCores.

Strategy (expert-parallel with F-split pairing):
  - Host computes the (tiny) routing: cosine scores -> softmax -> top-k ->
    renormalized gate weights. ~34 MFLOP, negligible vs the 34 GFLOP FFN.
  - Experts are sorted by token count and paired heavy/light. Core pair
    (2k, 2k+1) both handle experts (H[k], L[k]); core 2k computes the
    first half of D_FF, core 2k+1 the second half. Each core therefore
    runs tokens(H[k]) + tokens(L[k]) through an F/2-wide FFN: all cores
    execute an identical instruction stream with capacities (C1, C2) =
    (max heavy count, max light count) -- near-perfect load balance vs.
    padding every core to the global max count.
  - The two F-halves of y = W2^T gelu(W1^T x + b1) + b2 are partial sums;
    b2 is added only in half 0. Each core scales its partial output by
    the token gate; the host scatter-adds everything.

Device layout per core (P = 128 partitions):
  GEMM1: hT[f, t] = sum_d W1h[d, f] * xT[d, t]   (W1 tiles stationary)
         -> Gelu(. + b1) on ScalarE, cast to bf16
  GEMM2: yT[d, t] = sum_f W2h[f, d] * hT[f, t]   (W2 tiles stationary)
         -> (. + b2) * gate on VectorE, fp32 out

All DRAM inputs are pre-arranged on the host into the exact SBUF layout
(partition-contiguous), so every DMA moves large contiguous per-partition
segments. DMA issue cost is ~0.65us per dma_start on a HWDGE sequencer
and the 16 DMA engines round-robin across live transfers with no
priority, so the issue order IS the bandwidth priority: x slot1 ->
W1 slot1 f-blocks -> x slot2 -> W1 slot2 -> W2. A run of dummy matmuls
on a zeroed tile warms the PE HAM clock (1.2 -> 2.4 GHz) and keeps the
PE busy until the first inputs land (idle gaps reset the p-state ramp).
"""

import numpy as np
import ml_dtypes

P = 128
D_MODEL = 1024
D_FF = 2048
N_EXPERTS = 8
N_CORES = 8
N_WARMUP_MM = 52

_BF16 = ml_dtypes.bfloat16

_cache: dict = {}
last_results = None  # BassKernelResults of the most recent run (for profiling)


def _chunks(C):
    out = []
    c0 = 0
    while c0 < C:
        cw = min(512, C - c0)
        out.append((c0, cw))
        c0 += cw
    return out


def _build(C1, C2):
    """Build + compile the SPMD paired-expert F-split FFN kernel."""
    import concourse.mybir as mybir
    from concourse import bacc
    from concourse.tile import TileContext

    D = D_MODEL
    ND = D // P          # 8 d-tiles
    NF1 = (D_FF // 2) // P  # 8 f-blocks per slot (F/2 = 1024)
    CS = [C1, C2]
    CK = [_chunks(C1), _chunks(C2)]

    nc = bacc.Bacc("TRN2", target_bir_lowering=False, debug=False,
                   enable_partition_id=False)

    # Host-pre-arranged layouts (see kernel() for the packing):
    #   xT:  [P, ND*(C1+C2)]  slot s at col ND*C1*s; inside: d*Cs + t
    #   w1:  [P, 2*NF1*ND*P]  slot s f-block f at (s*NF1+f)*ND*P + d*P + j
    #   w2:  [P, 2*NF1*D]     slot s f-tile f at (s*NF1+f)*D + j
    #   meta:[P, 2*NF1 + 2*ND + C1 + C2] = b1 halves | b2 halves | gates
    xT_d = nc.dram_tensor("xT", [P, ND * (C1 + C2)], mybir.dt.bfloat16,
                          kind="ExternalInput")
    w1_d = nc.dram_tensor("w1", [P, 2 * NF1 * ND * P], mybir.dt.bfloat16,
                          kind="ExternalInput")
    w2_d = nc.dram_tensor("w2", [P, 2 * NF1 * D], mybir.dt.bfloat16,
                          kind="ExternalInput")
    MW = 2 * NF1 + 2 * ND + C1 + C2
    meta_d = nc.dram_tensor("meta", [P, MW], mybir.dt.float32,
                            kind="ExternalInput")
    out_d = nc.dram_tensor("out", [D, C1 + C2], mybir.dt.float32,
                           kind="ExternalOutput")

    OX = [0, ND * C1]        # xT col offset per slot
    OH = [0, NF1 * C1]       # ht col offset per slot
    OG = [2 * NF1 + 2 * ND, 2 * NF1 + 2 * ND + C1]  # gate col offset
    OO = [0, C1]             # out col offset per slot

    with TileContext(nc) as tc:
        with (
            tc.tile_pool(name="weights", bufs=1) as wp,
            tc.tile_pool(name="acts", bufs=1) as ap,
            tc.tile_pool(name="outs", bufs=4) as op,
            tc.tile_pool(name="psum", bufs=2, space="PSUM") as pp,
        ):
            # --- PE warm-up: dummy matmuls on a zeroed tile, no DMA deps.
            # The junk accumulator shares chunk 0's psum slot (used first).
            dummy = ap.tile([P, P], mybir.dt.bfloat16, tag="dummy")
            nc.vector.memset(dummy[:], 0.0)
            wps = pp.tile([P, P], mybir.dt.float32, tag="ps1_0", name="warm_ps",
                          bufs=2)
            for _ in range(N_WARMUP_MM):
                nc.tensor.matmul(wps[:], dummy[:], dummy[:], start=True, stop=True)

            xt = ap.tile([P, ND * (C1 + C2)], mybir.dt.bfloat16, tag="xt")
            w1t = wp.tile([P, 2 * NF1 * ND * P], mybir.dt.bfloat16, tag="w1")
            w2t = wp.tile([P, 2 * NF1 * D], mybir.dt.bfloat16, tag="w2")
            mt = wp.tile([P, MW], mybir.dt.float32, tag="meta")
            b1t = mt[:, 0 : 2 * NF1]
            b2t = mt[:, 2 * NF1 : 2 * NF1 + 2 * ND]
            ht = ap.tile([P, NF1 * (C1 + C2)], mybir.dt.bfloat16, tag="ht")

            # --- DMAs, in strict bandwidth-priority order on sync; the
            # scalar HWDGE queue only fronts the small meta tensor.
            W1B = ND * P  # columns per W1 f-block
            XH1 = (ND // 2) * C1
            nc.sync.dma_start(out=xt[:, :XH1], in_=xT_d[:, :XH1])
            nc.sync.dma_start(out=w1t[:, :W1B], in_=w1_d[:, :W1B])
            nc.sync.dma_start(out=xt[:, XH1 : ND * C1],
                              in_=xT_d[:, XH1 : ND * C1])
            for f in range(1, NF1):
                nc.sync.dma_start(out=w1t[:, f * W1B : (f + 1) * W1B],
                                  in_=w1_d[:, f * W1B : (f + 1) * W1B])
            X2 = ND * C1 + ND * C2
            nc.sync.dma_start(out=xt[:, ND * C1 : X2], in_=xT_d[:, ND * C1 : X2])
            for f in range(NF1, 2 * NF1):
                nc.sync.dma_start(out=w1t[:, f * W1B : (f + 1) * W1B],
                                  in_=w1_d[:, f * W1B : (f + 1) * W1B])
            nc.scalar.dma_start(out=mt[:], in_=meta_d[:])
            NW2 = 4
            w2step = (2 * NF1 // NW2) * D
            for i in range(NW2):
                nc.sync.dma_start(out=w2t[:, i * w2step : (i + 1) * w2step],
                                  in_=w2_d[:, i * w2step : (i + 1) * w2step])

            # --- GEMM1 + GELU per slot: hT[f*P:(f+1)*P, t].
            # Chunk loop inside the d-accumulation: each W1 stationary tile
            # serves all chunks; LDWEIGHTS hides under the wide chunk.
            for s in range(2):
                Cs, ck = CS[s], CK[s]
                for f in range(NF1):
                    fb = s * NF1 + f
                    ps = [pp.tile([P, cw], mybir.dt.float32, tag=f"ps1_{ci}",
                                  name=f"ps1_{fb}_{ci}", bufs=2)
                          for ci, (c0, cw) in enumerate(ck)]
                    for d in range(ND):
                        lhs = w1t[:, fb * W1B + d * P : fb * W1B + (d + 1) * P]
                        for ci, (c0, cw) in enumerate(ck):
                            nc.tensor.matmul(
                                ps[ci][:],
                                lhs,
                                xt[:, OX[s] + d * Cs + c0 : OX[s] + d * Cs + c0 + cw],
                                start=(d == 0),
                                stop=(d == ND - 1),
                            )
                    for ci, (c0, cw) in enumerate(ck):
                        nc.scalar.activation(
                            ht[:, OH[s] + f * Cs + c0 : OH[s] + f * Cs + c0 + cw],
                            ps[ci][:],
                            mybir.ActivationFunctionType.Gelu,
                            bias=b1t[:, fb : fb + 1],
                        )

            # --- GEMM2 + bias + gate per slot: yT[do*P:(do+1)*P, t].
            for s in range(2):
                Cs, ck = CS[s], CK[s]
                for do in range(ND):
                    ps2 = [pp.tile([P, cw], mybir.dt.float32, tag=f"ps2_{ci}",
                                   name=f"ps2_{s}_{do}_{ci}", bufs=2 if ci == 0 else 1)
                           for ci, (c0, cw) in enumerate(ck)]
                    for f in range(NF1):
                        fb = s * NF1 + f
                        lhs = w2t[:, fb * D + do * P : fb * D + (do + 1) * P]
                        for ci, (c0, cw) in enumerate(ck):
                            nc.tensor.matmul(
                                ps2[ci][:],
                                lhs,
                                ht[:, OH[s] + f * Cs + c0 : OH[s] + f * Cs + c0 + cw],
                                start=(f == 0),
                                stop=(f == NF1 - 1),
                            )
                    ot = op.tile([P, Cs], mybir.dt.float32, tag="ot",
                                 name=f"ot_{s}_{do}")
                    last = s == 1 and do == ND - 1
                    # Tail-latency trim: for the final block, drain narrow
                    # chunks first and split wide chunks across both HWDGE
                    # engines so the last exposed transfer is small.
                    order = list(enumerate(ck))
                    if last:
                        order = order[::-1]
                    for ci, (c0, cw) in order:
                        if last and cw > 256:
                            pieces = [(c0, cw // 2, nc.sync),
                                      (c0 + cw // 2, cw - cw // 2, nc.scalar)]
                        else:
                            pieces = [(c0, cw, nc.scalar if last else nc.sync)]
                        for p0, pw, eng in pieces:
                            nc.vector.scalar_tensor_tensor(
                                ot[:, p0 : p0 + pw],
                                ps2[ci][:, p0 - c0 : p0 - c0 + pw],
                                b2t[:, s * ND + do : s * ND + do + 1],
                                mt[:, OG[s] + p0 : OG[s] + p0 + pw],
                                op0=mybir.AluOpType.add,
                                op1=mybir.AluOpType.mult,
                            )
                            eng.dma_start(
                                out=out_d[do * P : (do + 1) * P,
                                          OO[s] + p0 : OO[s] + p0 + pw],
                                in_=ot[:, p0 : p0 + pw],
                            )

    nc.compile()
    return nc


def _get_kernel(C1, C2):
    if (C1, C2) not in _cache:
        _cache[(C1, C2)] = _build(C1, C2)
    return _cache[(C1, C2)]


def _run_spmd(nc, in_maps):
    """run_bass_kernel_spmd, robust to a BASS_TRACE env the image can't
    serve (missing antenv.axon_hooks / artifact upload): install a best-
    effort NTFF hook shim, and on a trace-path failure fall back to an
    untraced run."""
    import os
    from concourse.bass_utils import run_bass_kernel_spmd

    try:
        import antenv.axon_hooks  # noqa: F401
    except ImportError:
        import sys
        import types
        hook = None
        try:
            from trn_agent_boot.trn_boot import _ntff_profile_via_ctypes
            hook = _ntff_profile_via_ctypes("/opt/axon/libaxon_pjrt.so")
        except Exception:
            hook = None
        mod = types.ModuleType("antenv.axon_hooks")
        mod.get_axon_ntff_profile_hook = lambda: hook
        try:
            import antenv
            antenv.axon_hooks = mod
            sys.modules["antenv.axon_hooks"] = mod
        except ImportError:
            pass

    core_ids = list(range(N_CORES))
    try:
        return run_bass_kernel_spmd(nc, in_maps, core_ids)
    except Exception:
        if os.environ.get("BASS_NEVER_TRACE") == "1":
            raise
        os.environ["BASS_NEVER_TRACE"] = "1"
        try:
            return run_bass_kernel_spmd(nc, in_maps, core_ids)
        finally:
            del os.environ["BASS_NEVER_TRACE"]


def _pack_w1_half(W1e, h, NF1, ND):
    w = np.asarray(W1e[:, h * (D_FF // 2) : (h + 1) * (D_FF // 2)],
                   dtype=np.float32).astype(_BF16)
    return np.ascontiguousarray(
        w.reshape(ND, P, NF1, P).transpose(1, 2, 0, 3).reshape(P, NF1 * ND * P))


def _pack_w2_half(W2e, h, NF1):
    w = np.asarray(W2e[h * (D_FF // 2) : (h + 1) * (D_FF // 2), :],
                   dtype=np.float32).astype(_BF16)
    return np.ascontiguousarray(
        w.reshape(NF1, P, D_MODEL).transpose(1, 0, 2).reshape(P, NF1 * D_MODEL))


def kernel(x, anchors, temperature, W1, b1, W2, b2, top_k):

    x = np.asarray(x)
    B, S, D = x.shape
    T = B * S
    E = np.asarray(anchors).shape[0]
    k = int(np.asarray(top_k))

    xf = np.ascontiguousarray(x.reshape(T, D), dtype=np.float32)

    # ---- routing on host (part of the dispatch decision) ----
    xn = xf / np.maximum(np.linalg.norm(xf, axis=-1, keepdims=True), 1e-8)
    an = np.asarray(anchors, dtype=np.float32)
    an = an / np.maximum(np.linalg.norm(an, axis=-1, keepdims=True), 1e-8)
    scores = (xn @ an.T) * abs(float(np.asarray(temperature)))
    scores -= scores.max(axis=-1, keepdims=True)
    probs = np.exp(scores)
    probs /= probs.sum(axis=-1, keepdims=True)
    topi = np.argsort(-probs, axis=-1, kind="stable")[:, :k]  # ties -> low idx
    topv = np.take_along_axis(probs, topi, axis=-1)
    gw = topv / (topv.sum(axis=-1, keepdims=True) + 1e-6)

    rows_per_e = []
    gates_per_e = []
    for e in range(E):
        mask = topi == e
        rows = np.nonzero(mask.any(axis=-1))[0]
        g = np.where(mask[rows], gw[rows], 0.0).sum(axis=-1).astype(np.float32)
        rows_per_e.append(rows)
        gates_per_e.append(g)

    # ---- pair heavy/light experts; 2 cores per pair split D_FF ----
    counts = np.array([len(r) for r in rows_per_e])
    order = np.argsort(-counts, kind="stable")
    heavy, light = order[: E // 2], order[E // 2 :]
    r32 = lambda n: max(64, -(-n // 32) * 32)
    C1 = r32(int(counts[heavy].max()))
    C2 = r32(int(counts[light].max()))
    nc = _get_kernel(C1, C2)

    ND, NF1 = D_MODEL // P, (D_FF // 2) // P
    x_bf = xf.astype(_BF16)
    CS = [C1, C2]

    def pack_x(dst, off, Cs, rows):
        n = len(rows)
        xv = dst[:, off : off + ND * Cs].reshape(P, ND, Cs)
        xv[:, :, :n] = x_bf[rows].reshape(n, ND, P).transpose(2, 1, 0)

    in_maps = []
    MW = 2 * NF1 + 2 * ND + C1 + C2
    for pair in range(E // 2):
        es = [int(heavy[pair]), int(light[pair])]
        # xT/meta gates are identical for both halves: build once.
        xT = np.zeros((P, ND * (C1 + C2)), dtype=_BF16)
        pack_x(xT, 0, C1, rows_per_e[es[0]])
        pack_x(xT, ND * C1, C2, rows_per_e[es[1]])
        for h in range(2):
            w1 = np.concatenate(
                [_pack_w1_half(np.asarray(W1[e]), h, NF1, ND) for e in es],
                axis=1)
            w2 = np.concatenate(
                [_pack_w2_half(np.asarray(W2[e]), h, NF1) for e in es], axis=1)
            meta = np.zeros((P, MW), dtype=np.float32)
            for s, e in enumerate(es):
                b1h = np.asarray(b1[e], dtype=np.float32)[
                    h * (D_FF // 2) : (h + 1) * (D_FF // 2)]
                meta[:, s * NF1 : (s + 1) * NF1] = b1h.reshape(NF1, P).T
                if h == 0:  # b2 contributes once per expert
                    meta[:, 2 * NF1 + s * ND : 2 * NF1 + (s + 1) * ND] = (
                        np.asarray(b2[e], dtype=np.float32).reshape(ND, P).T)
                g0 = 2 * NF1 + 2 * ND + (C1 if s else 0)
                meta[:, g0 : g0 + len(rows_per_e[e])] = gates_per_e[e][None, :]
            in_maps.append({"xT": xT, "w1": w1, "w2": w2, "meta": meta})

    res = _run_spmd(nc, in_maps)
    global last_results
    last_results = res

    # ---- combine (scatter-add the gated partial expert outputs) ----
    out = np.zeros((T, D_MODEL), dtype=np.float32)
    for pair in range(E // 2):
        es = [int(heavy[pair]), int(light[pair])]
        for h in range(2):
            o = res.results[2 * pair + h]["out"]
            for s, e in enumerate(es):
                rows = rows_per_e[e]
                n = len(rows)
                if n:
                    o0 = C1 if s else 0
                    out[rows] += o[:, o0 : o0 + n].T
    return out.reshape(B, S, D_MODEL)
